# revision 6
# baseline (speedup 1.0000x reference)
"""GraphSAGE link predictor on 8 Trainium2 NeuronCores (Bass/Tile).

Strategy (graph/data parallel, hardcoded from the sharding hint):
- Nodes are sharded contiguously across 8 cores (12500 real -> 12544 padded
  per core, 98 tiles of 128). Edges are sharded by dst node, sorted by
  (dst tile, src chunk), padded per (tile, chunk) cell to Bc 128-edge blocks
  (uniform across cores for SPMD). All feature data on device is bf16; PSUM
  accumulation and logits are fp32.
- The full row-major h replica lives in DRAM (layer 0: host-supplied x,
  later: AllGather of per-core slabs). Because dma_gather indices are int16,
  the replica is addressed in 4 chunks of NPAD/4 rows.
- Per layer, per group of G=7 dst tiles, per chunk c: ONE dma_gather pulls
  all G*Bc*128 source rows of that (group, chunk) into SBUF (dummy slots
  gather row 0); one DVE is_equal op builds the scaled one-hot for the phase;
  then G*Bc matmuls with lhsT=row-block, rhs=one-hot-block accumulate
  aggT[feat, dst] per tile in PSUM (accumulators stay open across the 4
  chunk phases). Degree normalization is folded into the PSUM->SBUF copy
  (DVE multiply by a replicated inv-degree tile).
- Combine per sub-chunk: hnextT = wn^T aggT + ws^T hT (+ hT residual),
  bias/relu on Scalar. Row-major slab rows are produced by an XBAR
  (dma_start_transpose) SBUF->SBUF transpose, DMA'd to the slab, and
  AllGather'd into the next layer's replica.
- Decoder: label edges sharded 25000/core, sorted by (src chunk, dst chunk)
  into 16 cells padded to LCB blocks; per cell one dma_gather for h3[src]
  and one for h3[dst], DVE multiply + per-block reduce -> logits [P, 16*LCB];
  the host inverts the cell permutation.
All weights ([128,128]) are replicated to every core.
"""

import numpy as np

import concourse.bass as bass
import concourse.bacc as bacc
import concourse.mybir as mybir
import concourse.tile as tile
from concourse.masks import make_identity

P = 128
D = 128
F32 = mybir.dt.float32
BF16 = mybir.dt.bfloat16
I16 = mybir.dt.int16
NCH = 4   # replica chunks (int16 index reach)
G = 2     # dst tiles per gather group


class Cfg:
    def __init__(self, C, NPCR, B, ELC):
        self.C = C                      # cores
        self.NPCR = NPCR                # real nodes per core
        self.TPC = (NPCR + P - 1) // P  # dst tiles per core
        self.NPC = self.TPC * P         # padded nodes per core
        self.NPAD = C * self.NPC        # padded total nodes
        self.B = B                      # edge blocks per dst tile (= NCH*Bc)
        self.Bc = B // NCH              # edge blocks per (tile, chunk) cell
        self.ELC = ELC                  # label edges per core (real)
        self.LBLK = (ELC + P - 1) // P  # label blocks per core
        self.LCB = None                 # label blocks per (schunk,dchunk) cell


def _wrap_idx(stream):
    """int16 index stream -> [128, len/16] wrapped (16 partitions x 8 Q7
    cores, replicated across the cores' bands)."""
    assert len(stream) % 16 == 0
    w16 = stream.reshape(-1, 16).T.astype(np.int16)
    return np.ascontiguousarray(np.tile(w16, (8, 1)))


def build_nc(cfg, n_layers=3, decoder=True, scratch=16384):
    C, TPC, NPC, NPAD, B, Bc, LBLK, LCB = (
        cfg.C, cfg.TPC, cfg.NPC, cfg.NPAD, cfg.B, cfg.Bc, cfg.LBLK, cfg.LCB)
    assert LCB is not None
    assert TPC % G == 0
    NG = TPC // G
    CH = NPAD // NCH
    GL = G * Bc * P          # idxs per (group, chunk) gather
    WE = NG * NCH * GL // 16  # idx cols for edge gathers
    LCAP = LCB * P           # label slots per cell
    WL = 16 * LCAP // 16     # idx cols for decoder gathers (each side)
    BD = B * D

    nc = bacc.Bacc("TRN2", target_bir_lowering=False, debug=False, num_devices=C,
                   dynamic_dma_scratch_size=scratch)

    # ---- I/O ----
    xck = [nc.dram_tensor(f"xc{i}", [CH, D], BF16, kind="ExternalInput")
           for i in range(NCH)]
    xt = nc.dram_tensor("xt", [P, NPC], BF16, kind="ExternalInput")
    idxe = nc.dram_tensor("idxe", [P, WE], I16, kind="ExternalInput")
    dloc = nc.dram_tensor("dloc", [P, TPC * B], BF16, kind="ExternalInput")
    invdrep = nc.dram_tensor("invdrep", [P, NPC], BF16, kind="ExternalInput")
    wn_d = nc.dram_tensor("wn", [3, D, D], BF16, kind="ExternalInput")
    ws_d = nc.dram_tensor("ws", [3, D, D], BF16, kind="ExternalInput")
    bias_d = nc.dram_tensor("bias", [3, D], F32, kind="ExternalInput")
    idxls = nc.dram_tensor("idxls", [P, WL], I16, kind="ExternalInput")
    idxld = nc.dram_tensor("idxld", [P, WL], I16, kind="ExternalInput")
    logits = nc.dram_tensor("logits", [P, 16 * LCB], F32,
                            kind="ExternalOutput")

    # internal DRAM: per-layer slabs + allgather replicas (bf16)
    slabs = [nc.dram_tensor(f"slab{l}", [NPC, D], BF16, kind="Internal")
             for l in range(3)]
    reps = [nc.dram_tensor(f"rep{l}", [NPAD, D], BF16, kind="Internal",
                           addr_space="Shared") for l in range(3)]
    rc = [[nc.dram_tensor(f"rc{l}_{i}", [CH, D], BF16, kind="Internal")
           for i in range(NCH)] for l in range(3)]

    with tile.TileContext(nc) as tc:
        with (
            tc.tile_pool(name="big", bufs=1) as bigp,
            tc.tile_pool(name="const", bufs=1) as cstp,
            tc.tile_pool(name="oh", bufs=1) as ohp,
            tc.tile_pool(name="xe", bufs=1) as xep,
            tc.tile_pool(name="aggsb", bufs=1) as asbp,
            tc.tile_pool(name="rows", bufs=1) as rowp,
            tc.tile_pool(name="psA", bufs=2 * G, space="PSUM") as psA,
            tc.tile_pool(name="psC", bufs=2, space="PSUM") as psC,
        ):
            # ---- constants / static data in SBUF ----
            hA = bigp.tile([P, NPC], BF16, name="hA")
            hB = bigp.tile([P, NPC], BF16, name="hB")
            invd_sb = bigp.tile([P, NPC], BF16, name="invd_sb")
            idxe_sb = cstp.tile([P, WE], I16, name="idxe_sb")
            dloc_sb = cstp.tile([P, TPC * B], BF16, name="dloc_sb")
            idxls_sb = cstp.tile([P, WL], I16, name="idxls_sb")
            idxld_sb = cstp.tile([P, WL], I16, name="idxld_sb")
            cst = cstp.tile([P, 8 * D], BF16, name="cst")
            bias_sb = cstp.tile([P, 3], F32, name="bias_sb")

            nc.sync.dma_start(out=hA[:], in_=xt.ap())
            nc.sync.dma_start(out=invd_sb[:], in_=invdrep.ap())
            nc.sync.dma_start(out=idxe_sb[:], in_=idxe.ap())
            nc.sync.dma_start(out=dloc_sb[:], in_=dloc.ap())
            nc.sync.dma_start(out=idxls_sb[:], in_=idxls.ap())
            nc.sync.dma_start(out=idxld_sb[:], in_=idxld.ap())

            ident = cst[:, 0:D]
            iota_f = cst[:, D:2 * D]
            wn_t = [cst[:, (2 + l) * D:(3 + l) * D] for l in range(3)]
            ws_t = [cst[:, (5 + l) * D:(6 + l) * D] for l in range(3)]
            make_identity(nc, ident)
            iota_i = cstp.tile([P, D], mybir.dt.int32, name="iota_i")
            nc.gpsimd.iota(iota_i[:], pattern=[[1, D]], base=0,
                           channel_multiplier=0)
            nc.vector.tensor_copy(iota_f, iota_i[:])
            for l in range(3):
                nc.sync.dma_start(out=wn_t[l], in_=wn_d.ap()[l])
                nc.sync.dma_start(out=ws_t[l], in_=ws_d.ap()[l])
                nc.sync.dma_start(out=bias_sb[:, l:l + 1],
                                  in_=bias_d.ap()[l][:, None])

            GBD = G * Bc * D
            oh_big = ohp.tile([P, 2 * GBD], BF16, name="oh_big")
            xe_big = xep.tile([P, 2 * GBD], BF16, name="xe_big")
            aggT = asbp.tile([P, 2 * G * P], BF16, name="aggT")
            rows_sb = rowp.tile([P, 2 * G * P], BF16, name="rows_sb")

            # ================= 3 GraphSAGE layers =================
            for l in range(n_layers):
                h_in = hA if l % 2 == 0 else hB
                h_out = hB if l % 2 == 0 else hA
                src_c = xck if l == 0 else rc[l - 1]
                relu = l < 2
                residual = l > 0

                for g in range(NG):
                    t0 = g * G
                    agg_ps = [psA.tile([P, D], F32, tag="agg",
                                       name=f"agg_{l}_{t0 + k}")
                              for k in range(G)]
                    for ch in range(NCH):
                        ph = g * NCH + ch
                        # ONE gather for the whole (group, chunk) phase
                        xe = xe_big[:, (ph % 2) * GBD:((ph % 2) + 1) * GBD]
                        for q0 in range(0, GL, 512):
                            qn = min(512, GL - q0)
                            nc.gpsimd.dma_gather(
                                out_ap=xe[:, (q0 // P) * D:
                                          ((q0 + qn) // P) * D]
                                    .rearrange("p (b d) -> p b d", d=D),
                                in_ap=src_c[ch].ap(),
                                idxs_ap=idxe_sb[:, ph * (GL // 16) + q0 // 16:
                                                ph * (GL // 16) + (q0 + qn) // 16],
                                num_idxs=qn,
                                num_idxs_reg=qn,
                                elem_size=D,
                            )
                        # scaled one-hot for the phase in one DVE op
                        oh = oh_big[:, (ph % 2) * GBD:((ph % 2) + 1) * GBD]
                        nc.vector.tensor_tensor(
                            out=oh.rearrange("p (b d) -> p b d", d=D),
                            in0=dloc_sb[:, ph * G * Bc:(ph + 1) * G * Bc]
                                [:, :, None].broadcast_to([P, G * Bc, D]),
                            in1=iota_f[:, None, :].broadcast_to([P, G * Bc, D]),
                            op=mybir.AluOpType.is_equal,
                        )
                        for k in range(G):
                            for bb in range(Bc):
                                j = k * Bc + bb
                                nc.tensor.matmul(
                                    out=agg_ps[k][:],
                                    lhsT=xe[:, j * D:(j + 1) * D],
                                    rhs=oh[:, j * D:(j + 1) * D],
                                    start=(ch == 0 and bb == 0),
                                    stop=(ch == NCH - 1 and bb == Bc - 1),
                                )
                    # mean-normalize on the PSUM->SBUF copy (DVE)
                    aggT_c = aggT[:, (g % 2) * G * P:((g % 2) + 1) * G * P]
                    for k in range(G):
                        t = t0 + k
                        nc.vector.tensor_tensor(
                            out=aggT_c[:, k * P:(k + 1) * P],
                            in0=agg_ps[k][:],
                            in1=invd_sb[:, t * P:(t + 1) * P],
                            op=mybir.AluOpType.mult,
                        )
                    # ---- combine in sub-chunks of <=4 tiles ----
                    for s0 in range(0, G, 4):
                        sn = min(4, G - s0)
                        cn = sn * P
                        cs = (t0 + s0) * P
                        cps = psC.tile([P, 512], F32, tag="comb",
                                       name=f"cb_{l}_{g}_{s0}")
                        rhs_a = aggT_c[:, s0 * P:s0 * P + cn]
                        nc.tensor.matmul(out=cps[:, :cn], lhsT=wn_t[l],
                                         rhs=rhs_a, start=True, stop=False)
                        nc.tensor.matmul(out=cps[:, :cn], lhsT=ws_t[l],
                                         rhs=h_in[:, cs:cs + cn],
                                         start=False, stop=not residual)
                        if residual:
                            nc.tensor.matmul(out=cps[:, :cn], lhsT=ident,
                                             rhs=h_in[:, cs:cs + cn],
                                             start=False, stop=True)
                        if relu:
                            nc.scalar.activation(
                                out=h_out[:, cs:cs + cn], in_=cps[:, :cn],
                                func=mybir.ActivationFunctionType.Relu,
                                bias=bias_sb[:, l:l + 1],
                            )
                        else:
                            nc.vector.tensor_scalar_add(
                                out=h_out[:, cs:cs + cn], in0=cps[:, :cn],
                                scalar1=bias_sb[:, l:l + 1],
                            )
                        # rows out to slab via XBAR transpose
                        rsb = rows_sb[:, (g % 2) * G * P + s0 * P:
                                      (g % 2) * G * P + s0 * P + cn]
                        nc.sync.dma_start_transpose(
                            out=rsb.rearrange("p (j d) -> p j d", d=D),
                            in_=h_out[:, cs:cs + cn])
                        nc.sync.dma_start(
                            out=slabs[l].ap()[cs:cs + cn, :]
                                .rearrange("(j p) d -> p j d", p=P),
                            in_=rsb.rearrange("p (j d) -> p j d", d=D))
                nc.gpsimd.collective_compute(
                    "AllGather", mybir.AluOpType.bypass,
                    replica_groups=[list(range(C))],
                    ins=[slabs[l].ap()], outs=[reps[l].ap()],
                )
                for ch in range(NCH):
                    nc.sync.dma_start(
                        out=rc[l][ch].ap(),
                        in_=reps[l].ap()[ch * CH:(ch + 1) * CH, :])

            # ================= link decoder =================
            sed = rowp.tile([P, 2 * LCB * D], BF16, name="sed")
            ded = rowp.tile([P, 2 * LCB * D], BF16, name="ded")
            prd = rowp.tile([P, 2 * LCB * D], BF16, name="prd")
            resL = rowp.tile([P, 16 * LCB], F32, name="resL")
            rc_last = rc[n_layers - 1]
            cells = range(16) if decoder else []
            if not decoder:
                nc.gpsimd.memset(resL[:], 0.0)
            for cell in cells:
                s, dch = cell // NCH, cell % NCH
                off = (cell % 2) * LCB * D
                se = sed[:, off:off + LCB * D]
                de = ded[:, off:off + LCB * D]
                pr = prd[:, off:off + LCB * D]
                for q0 in range(0, LCAP, 512):
                    qn = min(512, LCAP - q0)
                    nc.gpsimd.dma_gather(
                        out_ap=se[:, (q0 // P) * D:((q0 + qn) // P) * D]
                            .rearrange("p (b d) -> p b d", d=D),
                        in_ap=rc_last[s].ap(),
                        idxs_ap=idxls_sb[:, cell * (LCAP // 16) + q0 // 16:
                                         cell * (LCAP // 16) + (q0 + qn) // 16],
                        num_idxs=qn, num_idxs_reg=qn, elem_size=D)
                    nc.gpsimd.dma_gather(
                        out_ap=de[:, (q0 // P) * D:((q0 + qn) // P) * D]
                            .rearrange("p (b d) -> p b d", d=D),
                        in_ap=rc_last[dch].ap(),
                        idxs_ap=idxld_sb[:, cell * (LCAP // 16) + q0 // 16:
                                         cell * (LCAP // 16) + (q0 + qn) // 16],
                        num_idxs=qn, num_idxs_reg=qn, elem_size=D)
                nc.vector.tensor_mul(out=pr, in0=se, in1=de)
                nc.vector.tensor_reduce(
                    out=resL[:, cell * LCB:(cell + 1) * LCB][:, :, None],
                    in_=pr.rearrange("p (g d) -> p g d", g=LCB),
                    axis=mybir.AxisListType.X, op=mybir.AluOpType.add)
            nc.sync.dma_start(out=logits.ap(), in_=resL[:])
    nc.compile()
    return nc


# ----------------------------------------------------------------------------
# host-side preprocessing
# ----------------------------------------------------------------------------

def compute_B(cfg, edge_index):
    """Uniform blocks per (dst tile, src chunk) cell * NCH."""
    src = np.asarray(edge_index[0], dtype=np.int64)
    dst = np.asarray(edge_index[1], dtype=np.int64)
    c_of = dst // cfg.NPCR
    loc = dst - c_of * cfg.NPCR
    tile_g = c_of * cfg.TPC + loc // P
    CH = cfg.NPAD // NCH
    src_pidx = (src // cfg.NPCR) * cfg.NPC + (src % cfg.NPCR)
    schunk = src_pidx // CH
    cell = tile_g * NCH + schunk
    counts = np.bincount(cell, minlength=cfg.C * cfg.TPC * NCH)
    return NCH * int((counts.max() + P - 1) // P)


def compute_LCB(cfg, edge_label_index):
    lsrc = np.asarray(edge_label_index[0], dtype=np.int64)
    ldst = np.asarray(edge_label_index[1], dtype=np.int64)
    CH = cfg.NPAD // NCH
    ls = ((lsrc // cfg.NPCR) * cfg.NPC + (lsrc % cfg.NPCR)) // CH
    ld = ((ldst // cfg.NPCR) * cfg.NPC + (ldst % cfg.NPCR)) // CH
    core = np.repeat(np.arange(cfg.C), cfg.ELC)
    cell = (core * 16) + ls * NCH + ld
    counts = np.bincount(cell, minlength=cfg.C * 16)
    return int((counts.max() + P - 1) // P)


def prep_inputs(cfg, node_features, edge_index, edge_label_index,
                w_neigh, w_self, bias):
    import ml_dtypes
    BF = ml_dtypes.bfloat16
    C, NPCR, TPC, NPC, NPAD, B, Bc, ELC = (
        cfg.C, cfg.NPCR, cfg.TPC, cfg.NPC, cfg.NPAD, cfg.B, cfg.Bc, cfg.ELC)
    if cfg.LCB is None:
        cfg.LCB = compute_LCB(cfg, edge_label_index)
    LCB = cfg.LCB
    LCAP = LCB * P
    N = node_features.shape[0]
    E = edge_index.shape[1]
    CH = NPAD // NCH
    NG = TPC // G

    src = np.asarray(edge_index[0], dtype=np.int64)
    dst = np.asarray(edge_index[1], dtype=np.int64)
    deg = np.bincount(dst, minlength=N).astype(np.float32)
    invdeg = 1.0 / np.maximum(deg, 1.0)

    c_of = dst // NPCR
    loc = dst - c_of * NPCR
    tile_g = c_of * TPC + loc // P
    src_pidx = (src // NPCR) * NPC + (src % NPCR)
    schunk = src_pidx // CH
    cell = tile_g * NCH + schunk
    order = np.argsort(cell, kind="stable")
    scell = cell[order]
    counts = np.bincount(scell, minlength=C * TPC * NCH)
    cap = Bc * P
    assert counts.max() <= cap, (counts.max(), cap)
    starts = np.zeros(C * TPC * NCH, np.int64)
    starts[1:] = np.cumsum(counts)[:-1]
    pos = np.arange(E) - starts[scell]
    slot = scell * cap + pos

    # IDX/DLOC over all slots: [C, TPC, NCH, Bc, P]
    IDX = np.zeros(C * TPC * NCH * cap, np.int16)
    DLOC = np.full(C * TPC * NCH * cap, -1.0, np.float32)
    IDX[slot] = (src_pidx[order] - schunk[order] * CH).astype(np.int16)
    DLOC[slot] = (loc[order] % P).astype(np.float32)
    IDX = IDX.reshape(C, TPC, NCH, Bc, P)
    DLOC = DLOC.reshape(C, TPC, NCH, Bc, P)

    # idx stream per core in (g, ch, k, bb, p) order, wrapped for dma_gather
    idxe_pc = []
    dloc_pc = []
    for c in range(C):
        st = IDX[c].reshape(NG, G, NCH, Bc, P).transpose(0, 2, 1, 3, 4)
        idxe_pc.append(_wrap_idx(st.reshape(-1)))
        dl = DLOC[c].reshape(NG, G, NCH, Bc, P).transpose(4, 0, 2, 1, 3)
        dloc_pc.append(np.ascontiguousarray(
            dl.reshape(P, TPC * B).astype(BF)))

    # padded x replica (bf16)
    x = np.asarray(node_features, dtype=np.float32)
    xpad = np.zeros((NPAD, D), BF)
    for c in range(C):
        xpad[c * NPC:c * NPC + NPCR] = x[c * NPCR:(c + 1) * NPCR].astype(BF)

    # invdrep per core: [P, NPC] bf16 (invdeg of local node, on every
    # partition)
    invdrep_pc = np.ones((C, P, NPC), np.float32)
    for c in range(C):
        v = np.ones(NPC, np.float32)
        v[:NPCR] = invdeg[c * NPCR:(c + 1) * NPCR]
        invdrep_pc[c] = np.broadcast_to(v[None, :], (P, NPC))

    # label edges: sort into 16 (schunk, dchunk) cells per core
    lsrc = np.asarray(edge_label_index[0], dtype=np.int64)
    ldst = np.asarray(edge_label_index[1], dtype=np.int64)
    lsrc_p = ((lsrc // NPCR) * NPC + (lsrc % NPCR)).astype(np.int64)
    ldst_p = ((ldst // NPCR) * NPC + (ldst % NPCR)).astype(np.int64)
    idxls_pc, idxld_pc, slot_perm = [], [], []
    for c in range(C):
        sl = lsrc_p[c * ELC:(c + 1) * ELC]
        dl = ldst_p[c * ELC:(c + 1) * ELC]
        cell_l = (sl // CH) * NCH + (dl // CH)
        order_l = np.argsort(cell_l, kind="stable")
        scl = cell_l[order_l]
        cnts = np.bincount(scl, minlength=16)
        assert cnts.max() <= LCAP, (cnts.max(), LCAP)
        st = np.zeros(16, np.int64)
        st[1:] = np.cumsum(cnts)[:-1]
        posl = np.arange(ELC) - st[scl]
        slotl = scl * LCAP + posl
        SRC = np.zeros(16 * LCAP, np.int16)
        DST = np.zeros(16 * LCAP, np.int16)
        SRC[slotl] = (sl[order_l] - (scl // NCH) * CH).astype(np.int16)
        DST[slotl] = (dl[order_l] - (scl % NCH) * CH).astype(np.int16)
        idxls_pc.append(_wrap_idx(SRC))
        idxld_pc.append(_wrap_idx(DST))
        inv = np.empty(ELC, np.int64)
        inv[order_l] = slotl
        slot_perm.append(inv)
    cfg.slot_perm = slot_perm

    wn = np.ascontiguousarray(np.asarray(w_neigh, dtype=np.float32)).astype(BF)
    ws = np.ascontiguousarray(np.asarray(w_self, dtype=np.float32)).astype(BF)
    bs = np.ascontiguousarray(np.asarray(bias, dtype=np.float32))

    in_maps = []
    for c in range(C):
        xtc = np.zeros((P, NPC), BF)
        xtc[:, :NPCR] = x[c * NPCR:(c + 1) * NPCR].T.astype(BF)
        in_maps.append({
            **{f"xc{i}": np.ascontiguousarray(
                xpad[i * (NPAD // NCH):(i + 1) * (NPAD // NCH)])
               for i in range(NCH)},
            "xt": xtc,
            "idxe": idxe_pc[c],
            "dloc": dloc_pc[c],
            "invdrep": invdrep_pc[c].astype(BF),
            "wn": wn, "ws": ws, "bias": bs,
            "idxls": idxls_pc[c],
            "idxld": idxld_pc[c],
        })
    return in_maps


# ----------------------------------------------------------------------------
# PJRT runner (inlined; kernel.py must be self-contained)
# ----------------------------------------------------------------------------

class _Runner:
    def __init__(self, nc, n_cores):
        import jax
        from jax.sharding import Mesh, PartitionSpec
        from jax.experimental.shard_map import shard_map
        from concourse import bass2jax
        from concourse.bass2jax import _bass_exec_p, install_neuronx_cc_hook

        install_neuronx_cc_hook()
        self.jax = jax
        self.n_cores = n_cores
        partition_name = (
            nc.partition_id_tensor.name if nc.partition_id_tensor else None)
        in_names, out_names, out_avals, zero_outs = [], [], [], []
        for alloc in nc.m.functions[0].allocations:
            if not isinstance(alloc, mybir.MemoryLocationSet):
                continue
            name = alloc.memorylocations[0].name
            if alloc.kind == "ExternalInput":
                if name != partition_name:
                    in_names.append(name)
            elif alloc.kind == "ExternalOutput":
                shape = tuple(alloc.tensor_shape)
                dtype = mybir.dt.np(alloc.dtype)
                out_names.append(name)
                out_avals.append(jax.core.ShapedArray(shape, dtype))
                zero_outs.append(np.zeros(shape, dtype))
        self.in_names, self.out_names = in_names, out_names
        self.out_avals, self.zero_outs = out_avals, zero_outs
        all_in = list(in_names) + list(out_names)
        if partition_name is not None:
            all_in.append(partition_name)

        def _body(*args):
            operands = list(args)
            if partition_name is not None:
                operands.append(bass2jax.partition_id_tensor())
            return tuple(_bass_exec_p.bind(
                *operands,
                out_avals=tuple(out_avals),
                in_names=tuple(all_in),
                out_names=tuple(out_names),
                lowering_input_output_aliases=(),
                sim_require_finite=True,
                sim_require_nnan=True,
                nc=nc,
            ))

        devices = jax.devices()[:n_cores]
        self.mesh = Mesh(np.asarray(devices), ("core",))
        n_outs = len(out_names)
        self.fn = jax.jit(
            shard_map(_body, mesh=self.mesh,
                      in_specs=(PartitionSpec("core"),) * (len(in_names) + n_outs),
                      out_specs=(PartitionSpec("core"),) * n_outs,
                      check_rep=False),
            keep_unused=True,
        )

    def stage(self, in_maps):
        from jax.sharding import NamedSharding, PartitionSpec
        concat = [np.concatenate([np.asarray(m[n]) for m in in_maps], axis=0)
                  for n in self.in_names]
        concat += [np.zeros((self.n_cores * z.shape[0], *z.shape[1:]), z.dtype)
                   for z in self.zero_outs]
        sh = NamedSharding(self.mesh, PartitionSpec("core"))
        staged = [self.jax.device_put(a, sh) for a in concat]
        self.jax.block_until_ready(staged)
        return staged

    def run_staged(self, staged):
        outs = self.fn(*staged)
        self.jax.block_until_ready(outs)
        return outs

    def split(self, outs):
        return [
            {n: np.asarray(outs[i]).reshape(self.n_cores,
                                            *self.out_avals[i].shape)[c]
             for i, n in enumerate(self.out_names)}
            for c in range(self.n_cores)
        ]


_CACHE = {}


def _get_runner(cfg_key, cfg):
    if cfg_key not in _CACHE:
        nc = build_nc(cfg)
        _CACHE[cfg_key] = _Runner(nc, cfg.C)
    return _CACHE[cfg_key]


def kernel(node_features, edge_index, edge_label_index, w_neigh, w_self,
           bias):
    node_features = np.asarray(node_features)
    edge_index = np.asarray(edge_index)
    edge_label_index = np.asarray(edge_label_index)
    N = node_features.shape[0]
    C = 8
    NPCR = N // C
    ELC = edge_label_index.shape[1] // C
    cfg = Cfg(C, NPCR, 20, ELC)
    B = compute_B(cfg, edge_index)
    if B > cfg.B:
        cfg = Cfg(C, NPCR, B, ELC)
    cfg.LCB = compute_LCB(cfg, edge_label_index)
    runner = _get_runner((C, NPCR, cfg.B, ELC, cfg.LCB), cfg)
    in_maps = prep_inputs(cfg, node_features, edge_index, edge_label_index,
                          w_neigh, w_self, bias)
    outs = runner.split(runner.run_staged(runner.stage(in_maps)))
    parts = []
    for c in range(C):
        flat = outs[c]["logits"].T.reshape(-1)
        parts.append(flat[cfg.slot_perm[c]])
    return np.concatenate(parts).astype(np.float32)
